# revision 5
# baseline (speedup 1.0000x reference)
"""Trainium2 Bass kernel for nn_CrossAttentionLayer (dual cross-attention
transformer layer).

Sharding: pure data parallel — B=8 batch elements, one per NeuronCore.
Each core computes the full layer for its batch element; no collectives.

On-chip layout: activations are kept transposed as [feature, token] so every
matmul contracts features on the partition dim, all biases / LN affines are
per-partition ops, and softmax statistics reduce along the free dim.

Matmuls run in bf16 (inputs rounded; accumulation fp32 in PSUM); softmax /
layernorm statistics are fp32. Attention-weight outputs are staged bf16 on
device and upcast on host.
"""

import os
import sys

sys.path.insert(0, "/opt/trn_rl_repo")

from contextlib import ExitStack

import numpy as np
import ml_dtypes

import concourse.bass as bass
import concourse.bacc as bacc
import concourse.tile as tile
from concourse import mybir

F32 = mybir.dt.float32
BF16 = mybir.dt.bfloat16
AF = mybir.ActivationFunctionType
ALU = mybir.AluOpType

P = 128
D = 512
L = 1024  # query tokens per batch element
S = 1024  # kv tokens
H = 8
HD = 64
DFF = 1024
DT = D // P    # 4 feature tiles
LT = L // P    # 8 token tiles
FT = DFF // P  # 8 ffn-hidden tiles
NB = 8         # batch == cores
EPS = 1e-5

BF = ml_dtypes.bfloat16


class _Consts:
    pass


def _load_consts(nc, tc, pool, cfg):
    """Small constants: identities / ones / selector / ln params / biases."""
    C = _Consts()

    def sb(name, shape, dtype):
        t = pool.tile(list(shape), dtype, tag=name, name=name)
        nc.sync.dma_start(out=t, in_=cfg["params"][name].ap())
        return t

    C.ident_f = sb("ident_f", [P, P], F32)
    C.ident_b = sb("ident_b", [P, P], BF16)
    C.ones_col_b = sb("ones_col_b", [P, 1], BF16)
    C.ones_1x128_f = sb("ones_1x128_f", [1, P], F32)
    C.sel_f = sb("sel_f", [LT, LT * HD], F32)
    C.one11_f = sb("one11_f", [1, 1], F32)
    for gname in ("g1", "g2", "g3"):
        setattr(C, gname + "_col", sb(gname + "_col", [P, DT], F32))
    for bname, width in cfg["bias_cols"].items():
        setattr(C, bname + "_col", sb(bname + "_col", [P, width], F32))
    return C


def _load_w(nc, pool, cfg, pname, dout, nk, tagp):
    """Load a [Din, dout] bf16 weight as nk tiles of [128, dout]."""
    tiles = []
    for k in range(nk):
        t = pool.tile([P, dout], BF16, tag=f"{tagp}{k}", name=f"{tagp}{k}")
        nc.sync.dma_start(out=t, in_=cfg["params"][pname].ap()[k * P:(k + 1) * P, :])
        tiles.append(t)
    return tiles


def _emit_transpose_in(nc, tc, C, x_dram, xT_f, xT_b):
    """DRAM x [L, D] f32 (natural) -> SBUF xT [D, L] f32 + bf16 shadow."""
    with ExitStack() as ectx:
        pool = ectx.enter_context(tc.tile_pool(name="tin_sb", bufs=5))
        pps = ectx.enter_context(tc.tile_pool(name="tin_ps", bufs=4, space="PSUM"))
        for g in range(2):  # groups of 4 token tiles
            nats = []
            for j in range(4):
                lt = g * 4 + j
                t = pool.tile([P, D], F32, tag="nat", name="nat")
                nc.sync.dma_start(out=t, in_=x_dram.ap()[lt * P:(lt + 1) * P, :])
                nats.append(t)
            for dt in range(DT):
                ps = pps.tile([P, 512], F32, tag="tp", name="tp")
                for j in range(4):
                    nc.tensor.transpose(
                        ps[:, j * P:(j + 1) * P], nats[j][:, dt * P:(dt + 1) * P],
                        C.ident_f,
                    )
                nc.vector.tensor_copy(xT_f[dt][:, g * 512:(g + 1) * 512], ps)
                nc.scalar.copy(xT_b[dt][:, g * 512:(g + 1) * 512], ps)


def _emit_transpose_out(nc, tc, C, yT_f, y_dram, tag):
    """SBUF yT [D, L] f32 -> DRAM y [L, D] f32 (natural)."""
    with ExitStack() as ectx:
        pool = ectx.enter_context(tc.tile_pool(name=f"tout_sb{tag}", bufs=3))
        pps = ectx.enter_context(
            tc.tile_pool(name=f"tout_ps{tag}", bufs=4, space="PSUM"))
        for lt in range(LT):
            ps = pps.tile([P, D], F32, tag="tp", name="tp")
            for dt in range(DT):
                nc.tensor.transpose(
                    ps[:, dt * P:(dt + 1) * P], yT_f[dt][:, lt * P:(lt + 1) * P],
                    C.ident_f,
                )
            nat = pool.tile([P, D], F32, tag="nat", name="nat")
            nc.vector.tensor_copy(nat, ps)
            nc.sync.dma_start(out=y_dram.ap()[lt * P:(lt + 1) * P, :], in_=nat)


def _emit_mha(nc, tc, C, cfg, idx, q_bf, kv_bf, resid_f, p_out, pre):
    """One cross-attention block. Writes pre-LN stream tiles (f32) to `pre`."""
    bq = getattr(C, f"bq{idx}_col", None)
    bk = getattr(C, f"bk{idx}_col", None)
    bo = getattr(C, f"bo{idx}_col", None)  # wout@bv + bout fold

    with ExitStack() as ectx:
        wpool = ectx.enter_context(tc.tile_pool(name=f"m{idx}w", bufs=1))
        qk_pool = ectx.enter_context(tc.tile_pool(name=f"m{idx}qk", bufs=1))
        epool = ectx.enter_context(tc.tile_pool(name=f"m{idx}e", bufs=1))
        spool = ectx.enter_context(tc.tile_pool(name=f"m{idx}s", bufs=2))
        pspool = ectx.enter_context(
            tc.tile_pool(name=f"m{idx}ps", bufs=1, space="PSUM"))

        wq = _load_w(nc, wpool, cfg, f"wq{idx}T", D, DT, "wq")
        wk = _load_w(nc, wpool, cfg, f"wk{idx}T", D, DT, "wk")
        wv = _load_w(nc, wpool, cfg, f"wv{idx}T", D, DT, "wv")
        wo = _load_w(nc, wpool, cfg, f"wo{idx}T", D, DT, "wo")

        qT = [qk_pool.tile([P, L], BF16, tag=f"qT{dt}", name=f"qT{dt}")
              for dt in range(DT)]
        kT = [qk_pool.tile([P, S], BF16, tag=f"kT{dt}", name=f"kT{dt}")
              for dt in range(DT)]
        v_sb = [qk_pool.tile([P, H, HD + 1], BF16, tag=f"v{st}", name=f"v{st}")
                for st in range(LT)]
        o_sb = [qk_pool.tile([P, L], BF16, tag=f"o{dt}", name=f"o{dt}")
                for dt in range(DT)]

        # ---- Q, K projections (output transposed [D, L])
        for dst, wT, src, bcol in ((qT, wq, q_bf, bq), (kT, wk, kv_bf, bk)):
            for m in range(DT):
                for g in range(2):
                    ps = pspool.tile([P, 512], F32, tag="aux", name="aux", bufs=2)
                    for k in range(DT):
                        nc.tensor.matmul(
                            ps,
                            lhsT=wT[k][:, m * P:(m + 1) * P],
                            rhs=src[k][:, g * 512:(g + 1) * 512],
                            start=(k == 0), stop=(k == DT - 1),
                        )
                    dslice = dst[m][:, g * 512:(g + 1) * 512]
                    if bcol is None:
                        nc.scalar.copy(dslice, ps)
                    else:
                        nc.vector.tensor_scalar_add(dslice, ps, bcol[:, m:m + 1])

        # ---- V projection (natural layout [s, dv]) + ones column for rowsums
        for st in range(LT):
            ps = pspool.tile([P, 512], F32, tag="aux", name="aux", bufs=2)
            for k in range(DT):
                nc.tensor.matmul(
                    ps,
                    lhsT=kv_bf[k][:, st * P:(st + 1) * P],
                    rhs=wv[k],
                    start=(k == 0), stop=(k == DT - 1),
                )
            nc.scalar.copy(
                v_sb[st][:, :, 0:HD], ps.rearrange("p (h e) -> p h e", h=H)
            )
            nc.vector.memset(v_sb[st][:, :, HD:HD + 1], 1.0)

        # ---- per-head attention
        for h in range(H):
            dt, base = h // 2, (h % 2) * HD
            expst = [epool.tile([P, L], BF16, tag=f"expst{st}",
                                name=f"expst{st}", bufs=2) for st in range(LT)]
            pv = pspool.tile([HD + 1, L], F32, tag="pv", name="pv")
            for st in range(LT):
                sps = pspool.tile([P, L], F32, tag="big", name="big", bufs=2)
                for g in range(2):
                    nc.tensor.matmul(
                        sps[:, g * 512:(g + 1) * 512],
                        lhsT=kT[dt][base:base + HD, st * P:(st + 1) * P],
                        rhs=qT[dt][base:base + HD, g * 512:(g + 1) * 512],
                        start=True, stop=True,
                    )
                # expST[s, l] = exp(S^T / 8), bf16
                nc.scalar.activation(expst[st], sps, AF.Exp, bias=0.0, scale=0.125)
                # accumulate O^T (rows 0..63) and rowsums (row 64)
                for g in range(2):
                    nc.tensor.matmul(
                        pv[:, g * 512:(g + 1) * 512],
                        lhsT=v_sb[st][:, h, :],
                        rhs=expst[st][:, g * 512:(g + 1) * 512],
                        start=(st == 0), stop=(st == LT - 1),
                    )

            # rowsums row -> recip columns [128, LT] and recip row [LT, 128]
            rs_sb = spool.tile([1, L], F32, tag="rs", name="rs")
            nc.vector.tensor_copy(rs_sb, pv[HD:HD + 1, :])
            rc_ps = pspool.tile([P, LT], F32, tag="aux", name="aux", bufs=2)
            for lt in range(LT):
                nc.tensor.matmul(
                    rc_ps[:, lt:lt + 1],
                    lhsT=rs_sb[0:1, lt * P:(lt + 1) * P],
                    rhs=C.one11_f,
                    start=True, stop=True,
                )
            recip_c = spool.tile([P, LT], F32, tag="recipc", name="recipc")
            nc.vector.reciprocal(recip_c, rc_ps)
            rr_ps = pspool.tile([LT, P], F32, tag="aux", name="aux", bufs=2)
            nc.tensor.transpose(rr_ps, recip_c, C.ident_f)
            rr_sb = spool.tile([LT, P], F32, tag="rr_sb", name="rr_sb")
            nc.vector.tensor_copy(rr_sb, rr_ps)

            # O normalize: o = (expS @ v) * recip[l]  (broadcast recip over dv)
            rep = pspool.tile([HD, L], F32, tag="big", name="big", bufs=2)
            for lt in range(LT):
                nc.tensor.matmul(
                    rep[:, lt * P:(lt + 1) * P],
                    lhsT=C.sel_f[:, lt * HD:(lt + 1) * HD],
                    rhs=rr_sb,
                    start=True, stop=True,
                )
            tmp_o = spool.tile([HD, L], BF16, tag="otmp", name="otmp")
            nc.scalar.copy(tmp_o, pv[0:HD, :])
            nc.vector.scalar_tensor_tensor(
                out=o_sb[dt][base:base + HD, :],
                in0=tmp_o, scalar=1.0, in1=rep,
                op0=ALU.mult, op1=ALU.mult,
            )

            # attention-weight output tiles: transpose expST -> [l, s], normalize
            for lt in range(LT):
                pt = pspool.tile([P, S], BF16, tag="big", name="big", bufs=2)
                for st in range(LT):
                    nc.tensor.transpose(
                        pt[:, st * P:(st + 1) * P],
                        expst[st][:, lt * P:(lt + 1) * P],
                        C.ident_b,
                    )
                p_sb = epool.tile([P, S], BF16, tag="p_sb", name="p_sb", bufs=3)
                nc.vector.tensor_scalar_mul(p_sb, pt, recip_c[:, lt:lt + 1])
                nc.sync.dma_start(
                    out=p_out.ap()[h, lt * P:(lt + 1) * P, :], in_=p_sb
                )

        # ---- out projection + residual -> pre-LN stream (f32)
        for m in range(DT):
            for g in range(2):
                ps = pspool.tile([P, 512], F32, tag="aux", name="aux", bufs=2)
                for k in range(DT):
                    nc.tensor.matmul(
                        ps,
                        lhsT=wo[k][:, m * P:(m + 1) * P],
                        rhs=o_sb[k][:, g * 512:(g + 1) * 512],
                        start=(k == 0), stop=(k == DT - 1),
                    )
                nc.vector.scalar_tensor_tensor(
                    out=pre[m][:, g * 512:(g + 1) * 512],
                    in0=ps,
                    scalar=(bo[:, m:m + 1] if bo is not None else 0.0),
                    in1=resid_f[m][:, g * 512:(g + 1) * 512],
                    op0=ALU.add, op1=ALU.add,
                )


def _emit_ln(nc, tc, C, name, pre_f, g_col, b_col, y_f, y_b):
    """y = g*(x - mu)*rstd (+ b) into y_f (f32) and optional y_b (bf16)."""
    with ExitStack() as ectx:
        wpool = ectx.enter_context(tc.tile_pool(name=f"{name}w", bufs=1))
        spool = ectx.enter_context(tc.tile_pool(name=f"{name}s", bufs=2))
        pps = ectx.enter_context(
            tc.tile_pool(name=f"{name}ps", bufs=1, space="PSUM"))

        # bf16 shadow of pre (for cheap stats matmuls) + squares
        preb = []
        xsq = []
        for dt in range(DT):
            tb = wpool.tile([P, L], BF16, tag=f"preb{dt}", name=f"preb{dt}")
            nc.scalar.copy(tb, pre_f[dt])
            preb.append(tb)
            tq = wpool.tile([P, L], BF16, tag=f"xsq{dt}", name=f"xsq{dt}")
            nc.scalar.activation(tq, tb, AF.Square, bias=0.0, scale=1.0)
            xsq.append(tq)

        sum_ps = pps.tile([1, L], F32, tag="row0", name="row0")
        sumsq_ps = pps.tile([1, L], F32, tag="row1", name="row1")
        for dst, src in ((sum_ps, preb), (sumsq_ps, xsq)):
            for g in range(2):
                for k in range(DT):
                    nc.tensor.matmul(
                        dst[:, g * 512:(g + 1) * 512],
                        lhsT=C.ones_col_b,
                        rhs=src[k][:, g * 512:(g + 1) * 512],
                        start=(k == 0), stop=(k == DT - 1),
                    )

        mu_row = spool.tile([1, L], F32, tag="mu", name="mu")
        nc.vector.tensor_scalar_mul(mu_row, sum_ps, 1.0 / D)
        musq = spool.tile([1, L], F32, tag="musq", name="musq")
        nc.vector.tensor_mul(musq, mu_row, mu_row)
        var_row = spool.tile([1, L], F32, tag="var", name="var")
        nc.vector.scalar_tensor_tensor(
            out=var_row, in0=sumsq_ps, scalar=1.0 / D, in1=musq,
            op0=ALU.mult, op1=ALU.subtract,
        )
        # rstd = exp(-0.5 * ln(var + eps))  (same ACT table set as attention exp)
        eps_t = spool.tile([1, 1], F32, tag="eps", name="eps")
        nc.vector.memset(eps_t, EPS)
        lnv = spool.tile([1, L], F32, tag="lnv", name="lnv")
        nc.scalar.activation(lnv, var_row, AF.Ln, bias=eps_t, scale=1.0)
        rstd_row = spool.tile([1, L], F32, tag="rstd", name="rstd")
        nc.scalar.activation(rstd_row, lnv, AF.Exp, bias=0.0, scale=-0.5)

        mu_rep = pps.tile([P, L], F32, tag="rep0", name="rep0")
        rstd_rep = pps.tile([P, L], F32, tag="rep1", name="rep1")
        for rep, row in ((mu_rep, mu_row), (rstd_rep, rstd_row)):
            for g in range(2):
                nc.tensor.matmul(
                    rep[:, g * 512:(g + 1) * 512],
                    lhsT=C.ones_1x128_f,
                    rhs=row[0:1, g * 512:(g + 1) * 512],
                    start=True, stop=True,
                )

        for dt in range(DT):
            t = wpool.tile([P, L], F32, tag="lnt", name="lnt", bufs=2)
            nc.vector.scalar_tensor_tensor(
                out=t, in0=pre_f[dt], scalar=1.0, in1=mu_rep,
                op0=ALU.mult, op1=ALU.subtract,
            )
            nc.vector.scalar_tensor_tensor(
                out=y_f[dt], in0=t, scalar=g_col[:, dt:dt + 1], in1=rstd_rep,
                op0=ALU.mult, op1=ALU.mult,
            )
            if b_col is not None:
                nc.vector.tensor_scalar_add(y_f[dt], y_f[dt], b_col[:, dt:dt + 1])
            if y_b is not None:
                nc.scalar.copy(y_b[dt], y_f[dt])


def _emit_ffn(nc, tc, C, cfg, idx, x_b, resid_f, pre):
    """relu(x@fw1.T + fb1)@fw2.T + fb2 + resid -> pre (f32 tiles)."""
    fb1 = getattr(C, "fb1_col", None)
    fb2 = getattr(C, "fb2_col", None)
    with ExitStack() as ectx:
        wpool = ectx.enter_context(tc.tile_pool(name=f"f{idx}w", bufs=1))
        pps = ectx.enter_context(
            tc.tile_pool(name=f"f{idx}ps", bufs=4, space="PSUM"))
        fw1 = _load_w(nc, wpool, cfg, "fw1T", DFF, DT, "fw1")
        fw2 = _load_w(nc, wpool, cfg, "fw2T", D, FT, "fw2")
        h_sb = [wpool.tile([P, L], BF16, tag=f"h{m}", name=f"h{m}")
                for m in range(FT)]
        for m in range(FT):
            for g in range(2):
                ps = pps.tile([P, 512], F32, tag="mm", name="mm")
                for k in range(DT):
                    nc.tensor.matmul(
                        ps,
                        lhsT=fw1[k][:, m * P:(m + 1) * P],
                        rhs=x_b[k][:, g * 512:(g + 1) * 512],
                        start=(k == 0), stop=(k == DT - 1),
                    )
                nc.scalar.activation(
                    h_sb[m][:, g * 512:(g + 1) * 512], ps, AF.Relu,
                    bias=(fb1[:, m:m + 1] if fb1 is not None else 0.0),
                    scale=1.0,
                )
        for m in range(DT):
            for g in range(2):
                ps = pps.tile([P, 512], F32, tag="mm", name="mm")
                for k in range(FT):
                    nc.tensor.matmul(
                        ps,
                        lhsT=fw2[k][:, m * P:(m + 1) * P],
                        rhs=h_sb[k][:, g * 512:(g + 1) * 512],
                        start=(k == 0), stop=(k == FT - 1),
                    )
                nc.vector.scalar_tensor_tensor(
                    out=pre[m][:, g * 512:(g + 1) * 512],
                    in0=ps,
                    scalar=(fb2[:, m:m + 1] if fb2 is not None else 0.0),
                    in1=resid_f[m][:, g * 512:(g + 1) * 512],
                    op0=ALU.add, op1=ALU.add,
                )


def build_program(bias_cols):
    """bias_cols: dict name -> width for nonzero bias columns to declare."""
    nc = bacc.Bacc(None, target_bir_lowering=False)
    params = {}

    def param(name, shape, dtype):
        params[name] = nc.declare_dram_parameter(name, list(shape), dtype, False)

    param("x1", [L, D], F32)
    param("x2", [L, D], F32)
    for i in (1, 2):
        for w in ("q", "k", "v", "o"):
            param(f"w{w}{i}T", [D, D], BF16)
    param("fw1T", [D, DFF], BF16)
    param("fw2T", [DFF, D], BF16)
    param("ident_f", [P, P], F32)
    param("ident_b", [P, P], BF16)
    param("ones_col_b", [P, 1], BF16)
    param("ones_1x128_f", [1, P], F32)
    param("sel_f", [LT, LT * HD], F32)
    param("one11_f", [1, 1], F32)
    for gname in ("g1", "g2", "g3"):
        param(gname + "_col", [P, DT], F32)
    for bname, width in bias_cols.items():
        param(bname + "_col", [P, width], F32)

    y1 = params["y1"] = nc.declare_dram_parameter("y1", [L, D], F32, True)
    y2 = params["y2"] = nc.declare_dram_parameter("y2", [L, D], F32, True)
    p1 = params["p1"] = nc.declare_dram_parameter("p1", [H, L, S], BF16, True)
    p2 = params["p2"] = nc.declare_dram_parameter("p2", [H, L, S], BF16, True)

    cfg = {"params": params, "bias_cols": bias_cols}

    with tile.TileContext(nc) as tc, ExitStack() as ctx:
        consts = ctx.enter_context(tc.tile_pool(name="consts", bufs=1))
        stream = ctx.enter_context(tc.tile_pool(name="stream", bufs=1))
        C = _load_consts(nc, tc, consts, cfg)

        def fslot(slot):
            return [stream.tile([P, L], F32, tag=f"{slot}_{dt}",
                                name=f"{slot}_{dt}") for dt in range(DT)]

        def bslot(slot):
            return [stream.tile([P, L], BF16, tag=f"{slot}_{dt}",
                                name=f"{slot}_{dt}") for dt in range(DT)]

        # f32 stream slots F0..F3 and bf16 shadow slots B0..B2, recycled
        x1T_f = fslot("F0")
        x2T_f = fslot("F1")
        x1T_b = bslot("B0")
        x2T_b = bslot("B1")
        _emit_transpose_in(nc, tc, C, params["x1"], x1T_f, x1T_b)
        _emit_transpose_in(nc, tc, C, params["x2"], x2T_f, x2T_b)

        pre1 = fslot("F2")
        _emit_mha(nc, tc, C, cfg, 1, x1T_b, x2T_b, x1T_f, params["p1"], pre1)
        x1p_f, x1p_b = fslot("F3"), bslot("B2")
        _emit_ln(nc, tc, C, "ln1", pre1, C.g1_col, getattr(C, "b1_col", None),
                 x1p_f, x1p_b)

        pre2 = fslot("F2")
        _emit_mha(nc, tc, C, cfg, 2, x2T_b, x1p_b, x2T_f, params["p2"], pre2)
        x2p_f, x2p_b = fslot("F0"), bslot("B0")
        _emit_ln(nc, tc, C, "ln2", pre2, C.g2_col, getattr(C, "b2_col", None),
                 x2p_f, x2p_b)

        pre3 = fslot("F1")
        _emit_ffn(nc, tc, C, cfg, 1, x1p_b, x1p_f, pre3)
        y1T = fslot("F2")
        _emit_ln(nc, tc, C, "ln3", pre3, C.g3_col, getattr(C, "b3_col", None),
                 y1T, None)
        pre4 = fslot("F3")
        _emit_ffn(nc, tc, C, cfg, 2, x2p_b, x2p_f, pre4)
        y2T = fslot("F1")
        _emit_ln(nc, tc, C, "ln4", pre4, C.g3_col, getattr(C, "b3_col", None),
                 y2T, None)

        _emit_transpose_out(nc, tc, C, y1T, y1, "a")
        _emit_transpose_out(nc, tc, C, y2T, y2, "b")

    nc.finalize()
    return nc


# ---------------------------------------------------------------- host side

_CACHE = {}
LAST_RESULT = None


def _col(v):
    """[D]-vector -> [128, D//128] column layout (column t = v[128t:128(t+1)])."""
    return np.ascontiguousarray(v.reshape(-1, P).T.astype(np.float32))


def _prep_inputs(inputs):
    f = lambda k: np.asarray(inputs[k], np.float32)
    win1, bin1 = f("win1"), f("bin1")
    win2, bin2 = f("win2"), f("bin2")
    wout1, bout1 = f("wout1"), f("bout1")
    wout2, bout2 = f("wout2"), f("bout2")
    fw1, fb1, fw2, fb2 = f("fw1"), f("fb1"), f("fw2"), f("fb2")

    common = {}
    for i, win, wout in ((1, win1, wout1), (2, win2, wout2)):
        common[f"wq{i}T"] = np.ascontiguousarray(win[0:D].T).astype(BF)
        common[f"wk{i}T"] = np.ascontiguousarray(win[D:2 * D].T).astype(BF)
        common[f"wv{i}T"] = np.ascontiguousarray(win[2 * D:3 * D].T).astype(BF)
        common[f"wo{i}T"] = np.ascontiguousarray(wout.T).astype(BF)
    common["fw1T"] = np.ascontiguousarray(fw1.T).astype(BF)
    common["fw2T"] = np.ascontiguousarray(fw2.T).astype(BF)

    common["ident_f"] = np.eye(P, dtype=np.float32)
    common["ident_b"] = np.eye(P, dtype=np.float32).astype(BF)
    common["ones_col_b"] = np.ones((P, 1), np.float32).astype(BF)
    common["ones_1x128_f"] = np.ones((1, P), np.float32)
    sel = np.zeros((LT, LT * HD), np.float32)
    for lt in range(LT):
        sel[lt, lt * HD:(lt + 1) * HD] = 1.0
    common["sel_f"] = sel
    common["one11_f"] = np.ones((1, 1), np.float32)
    common["g1_col"] = _col(f("g1"))
    common["g2_col"] = _col(f("g2"))
    common["g3_col"] = _col(f("g3"))

    bias_cols = {}

    def maybe_bias(name, v, width):
        if np.any(v != 0):
            bias_cols[name] = width
            common[name + "_col"] = _col(v)

    maybe_bias("bq1", bin1[0:D], DT)
    maybe_bias("bk1", bin1[D:2 * D], DT)
    maybe_bias("bq2", bin2[0:D], DT)
    maybe_bias("bk2", bin2[D:2 * D], DT)
    # v-bias folds through attention (softmax rows sum to 1) into out-proj bias
    bo1 = wout1 @ bin1[2 * D:3 * D] + bout1
    bo2 = wout2 @ bin2[2 * D:3 * D] + bout2
    maybe_bias("bo1", bo1, DT)
    maybe_bias("bo2", bo2, DT)
    maybe_bias("fb1", fb1, FT)
    maybe_bias("fb2", fb2, DT)
    maybe_bias("b1", f("b1"), DT)
    maybe_bias("b2", f("b2"), DT)
    maybe_bias("b3", f("b3"), DT)
    return common, bias_cols


def get_program_and_maps(inputs):
    common, bias_cols = _prep_inputs(inputs)
    key = tuple(sorted(bias_cols.items()))
    if key not in _CACHE:
        _CACHE[key] = build_program(bias_cols)
    nc = _CACHE[key]

    x1 = np.asarray(inputs["x1"], np.float32)
    x2 = np.asarray(inputs["x2"], np.float32)
    in_maps = []
    for c in range(NB):
        m = dict(common)
        m["x1"] = np.ascontiguousarray(x1[c])
        m["x2"] = np.ascontiguousarray(x2[c])
        in_maps.append(m)
    return nc, in_maps


def kernel(**inputs):
    global LAST_RESULT
    from concourse.bass_utils import run_bass_kernel_spmd

    nc, in_maps = get_program_and_maps(inputs)
    trace = os.environ.get("KERNEL_TRACE", "0") == "1"
    res = run_bass_kernel_spmd(nc, in_maps, list(range(NB)), trace=trace)
    LAST_RESULT = res
    r = res.results
    y1 = np.stack([r[c]["y1"] for c in range(NB)]).astype(np.float32)
    y2 = np.stack([r[c]["y2"] for c in range(NB)]).astype(np.float32)
    w12 = np.stack([np.asarray(r[c]["p1"]).astype(np.float32) for c in range(NB)])
    w21 = np.stack([np.asarray(r[c]["p2"]).astype(np.float32) for c in range(NB)])
    return (y1, y2, w12, w21)
